# revision 60
# baseline (speedup 1.0000x reference)
"""Trainium2 Bass kernel for nn_CheckinEncoder (2-layer GCN, PReLU between).

Math (per GCNConv layer, PyG semantics):
    deg[d]  = sum_{e: dst_e=d} w_e + 1                (weighted in-degree + self loop)
    dis     = deg^{-1/2}
    norm_e  = dis[src_e] * w_e * dis[dst_e]           (self loop: 1/deg[d])
    agg     = scatter_add(norm_e * x[src_e] -> dst_e) (aggregate-first; linear
    h       = agg @ W.T + b                            and aggregation commute)

Sharding: dst nodes row-partitioned across 8 cores (6250 each). Each core
owns the edges into its nodes. Layer 1 gathers per-edge feature rows from a
replicated fp16 copy of x; between layers h is replicated via TWO
AllGathers into Shared-output DRAM tensors (A = tiles [0,28) fired
mid-layer-1 so it overlaps remaining compute, B at the end; single writer
per Shared tensor is required for the fast direct peer-write path). Each
section's row count is < 32768, so sections double as the two int16 gather
windows for layer 2 (layer 1 splits x at row 32768 instead).

Aggregation runs on the TensorEngine: for each 128-edge chunk a selection
matrix S[e, j] = (j == dst_local_e) * norm_e is built on the VectorEngine
(iota compare) and agg[dst, f] += S.T @ G accumulates in PSUM; a PE
transpose then yields the dense matmul's lhsT. The bias is folded into the
dense PSUM chain as a rank-1 (ones x b) matmul, and PReLU is one fused DVE
op after an ACT PSUM->SBUF copy.

Self loops never enter the gather: their contribution dis^2[i]*feat[i]
reads contiguous own rows (layer 1: strided DMA of the core's x slab;
layer 2: the persistent SBUF hown buffer written by layer 1) and is added
into the aggregation PSUM with one fused DVE op per tile.

Gather slots are padded to fixed per-(tile,half) chunk counts, but the
padding is never transferred: trailing idx slots hold -1 and the true
(x16-rounded) count is loaded into a GPSIMD register per call
(num_idxs_reg) — the DGE stops at the first negative index. The first
GBUFS rotations of the gather pool are memset so the untouched slots stay
finite (S weight 0 * NaN would poison the matmul).
"""

import numpy as np

# ---------------------------------------------------------------- problem dims
N_NODES = 50000
IN_CH = 256
HID = 512
N_CORES = 8
P = 128
LO_SPLIT = 32768
FP16 = True  # gather/matmul datapath dtype (fp32 accumulation throughout)
NQ = 4       # SWDGE queues used round-robin for gathers
WORK_BUFS = 3  # work tile pool depth (pipeline overlap)
GT1 = 1      # tiles per gather group (single_packet caps a call at 1008 idxs)
GT2 = 1
SHARED_HFULL = True  # Shared-output AllGather (direct peer writes)
GBUFS = 4    # gather tile buffers (in-flight gather depth)
NEG_PAD = True  # pad idx with -1: DGE stops at first negative -> no traffic
TINY_GATHER = False  # ablation: clamp gather counts to 16 rows (no real traffic)


# ------------------------------------------------------------------ tile patch
# This container's walrus accepts at most 1 sync wait per instruction
# (2 for EventSemaphore); Tile can emit more. Two fixes: split the kernel-tail
# drain's waits across nops, and legalize the final BIR by hoisting excess
# waits onto inserted NoOps (same engine, just before the instruction).
_PATCHED = False


def _apply_patches():
    global _PATCHED
    if _PATCHED:
        return
    _PATCHED = True
    import concourse.mybir as mybir
    import concourse.tile as tile
    import concourse.bass2jax as bass2jax
    import concourse.bass_utils as bass_utils
    from concourse.vector_clock import ScopedClock

    def _patched_drain_and_barrier(self, tick_clock, wait_clock):
        nc = self.nc
        drain_inst = nc.sync.drain()
        wait_clock.add_sem_waits(
            drain_inst.ins, ScopedClock({None: tick_clock.global_clock})
        )
        waits = list(drain_inst.ins.sync_info.on_wait)
        if len(waits) > 1:
            drain_inst.ins.sync_info.on_wait = waits[:1]
            for w in waits[1:]:
                nop = nc.sync.nop(nofuse=True, hint="drain_split_wait")
                if nop.ins.sync_info is None:
                    nop.ins.sync_info = mybir.SyncInfo(on_wait=[w], on_update=[])
                else:
                    nop.ins.sync_info.on_wait = [w]
        nc.all_engine_barrier()
        assert self.sems is not None
        popped = nc._tile_sem_poison_stack.pop()
        assert popped is self._sem_poison
        nc.clear_and_free_semaphores(list(self.sems.allocated().values()))
        nc.all_engine_barrier()

    tile.TileContext._drain_and_barrier = _patched_drain_and_barrier

    def _legalize_bir_json(bir_bytes):
        import orjson

        m = orjson.loads(bir_bytes)
        for fn in m.get("functions", []):
            for blk in fn.get("blocks", []):
                out = []
                for inst in blk.get("instructions", []):
                    si = inst.get("sync_info")
                    cap = 2 if inst.get("opcode") == "EventSemaphore" else 1
                    if si and len(si.get("on_wait") or []) > cap:
                        waits = si["on_wait"]
                        for k, w in enumerate(waits[:-cap]):
                            out.append(
                                {
                                    "debug": inst.get("debug", 0),
                                    "engine": inst["engine"],
                                    "ins": [],
                                    "outs": [],
                                    "name": f"{inst['name']}-lw{k}",
                                    "opcode": "NoOp",
                                    "sync_info": {"on_update": [], "on_wait": [w]},
                                }
                            )
                        si["on_wait"] = waits[-cap:]
                    out.append(inst)
                blk["instructions"] = out
        return orjson.dumps(m)

    orig = bass_utils.compile_bir_kernel

    def _wrapped(bir_json, tmpdir, neff_name="file.neff", **kw):
        return orig(_legalize_bir_json(bir_json), tmpdir, neff_name, **kw)

    bass_utils.compile_bir_kernel = _wrapped
    bass2jax.compile_bir_kernel = _wrapped


# ------------------------------------------------------------ host preprocessing
def _edge_buckets(edge_index, edge_weight, n_nodes, n_cores):
    """Bucket (src, dst_local, norm) per (core, tile). Self loops are NOT
    included: their contribution dis^2[i] * x[i] uses contiguous own rows and
    is applied on-device without the gather (see selfw)."""
    npc = n_nodes // n_cores
    tiles = (npc + P - 1) // P

    src = np.asarray(edge_index[0], dtype=np.int64)
    dst = np.asarray(edge_index[1], dtype=np.int64)
    w = np.asarray(edge_weight, dtype=np.float32)

    deg = np.bincount(dst, weights=w.astype(np.float64), minlength=n_nodes)
    deg = deg.astype(np.float32) + 1.0  # + self loop weight
    dis = 1.0 / np.sqrt(deg)
    norm = (dis[src] * w * dis[dst]).astype(np.float32)

    core_of = dst // npc
    tile_of = (dst % npc) // P
    dloc = (dst % npc) % P

    per_ct = {}
    for c in range(n_cores):
        cm = core_of == c
        for t in range(tiles):
            m = cm & (tile_of == t)
            per_ct[(c, t)] = (src[m], dloc[m], norm[m])
    selfw = dis * dis  # self-loop weight per node
    return per_ct, tiles, npc, selfw


def _pack(per_ct, n_cores, tiles, srcmap, lo_split, gt):
    """Pack bucketed edges into fixed-shape gather idx + (dst,norm) meta
    arrays. `srcmap` maps original src node id -> gather row id.

    idx layout is GROUP-major (gt tiles per gather group): for each group,
    all member tiles' lo indices are contiguous (CL*P slots per tile), then
    all hi indices (CH*P per tile) — one dma_gather per (group, half).
    meta stays per-tile: cols 2*(t*cpt+ch) with lo chunks first."""
    max_lo, max_hi = 1, 0
    split = {}
    for key, (s, d, n) in per_ct.items():
        g = srcmap[s] if srcmap is not None else s
        order = np.argsort(g, kind="stable")
        g, d2, n2 = g[order], d[order], n[order]
        lo = g < lo_split
        split[key] = (g[lo], d2[lo], n2[lo], g[~lo] - lo_split, d2[~lo], n2[~lo])
        max_lo = max(max_lo, int(lo.sum()))
        max_hi = max(max_hi, int((~lo).sum()))

    CL = (max_lo + P - 1) // P
    CH = (max_hi + P - 1) // P
    cpt = CL + CH

    idx_arrays, meta_arrays, cnt_arrays = [], [], []
    pad_idx = -1 if NEG_PAD else 0
    for c in range(n_cores):
        idx_flat = np.full((tiles * cpt * P,), pad_idx, np.int16)
        cnt = np.zeros((1, tiles * 2), np.int32)
        meta = np.zeros((P, tiles * cpt * 2), np.float32)
        for t0 in range(0, tiles, gt):
            t1 = min(t0 + gt, tiles)
            gbase = t0 * cpt * P
            lo_sz = (t1 - t0) * CL * P
            for t in range(t0, t1):
                ti = t - t0
                slo, dlo, nlo, shi, dhi, nhi = split[(c, t)]
                lo_off = gbase + ti * CL * P
                hi_off = gbase + lo_sz + ti * CH * P
                # round real counts up to x16 (DGE walks 16 lanes); pad the
                # rounded range with idx 0 (real gathers), -1 beyond (skipped)
                nlo16 = min(max((len(slo) + 15) // 16 * 16, 16), CL * P)
                nhi16 = min(max((len(shi) + 15) // 16 * 16, 16), CH * P)
                idx_flat[lo_off : lo_off + nlo16] = 0
                idx_flat[hi_off : hi_off + nhi16] = 0
                idx_flat[lo_off : lo_off + len(slo)] = slo.astype(np.int16)
                idx_flat[hi_off : hi_off + len(shi)] = shi.astype(np.int16)
                cnt[0, 2 * t] = 16 if TINY_GATHER else nlo16
                cnt[0, 2 * t + 1] = 16 if TINY_GATHER else nhi16
                dcol = np.zeros((cpt * P,), np.float32)
                ncol = np.zeros((cpt * P,), np.float32)
                dcol[: len(dlo)] = dlo
                ncol[: len(nlo)] = nlo
                dcol[CL * P : CL * P + len(dhi)] = dhi
                ncol[CL * P : CL * P + len(nhi)] = nhi
                for ch in range(cpt):
                    m = t * cpt + ch
                    meta[:, 2 * m] = dcol[ch * P : (ch + 1) * P]
                    meta[:, 2 * m + 1] = ncol[ch * P : (ch + 1) * P]
        idxw = idx_flat.reshape(-1, 16).T
        idx_arrays.append(np.tile(idxw, (8, 1)).copy())
        meta_arrays.append(meta)
        cnt_arrays.append(cnt)
    return idx_arrays, meta_arrays, cnt_arrays, CL, CH


# AllGather split: h is replicated via TWO AllGathers into two Shared-output
# DRAM tensors (single writer each — required for the fast shared-output
# collective path). Section A = tiles [0, SPLIT_T) fires mid-layer-1 and
# overlaps the rest of layer-1 compute; B fires at the end. Both sections'
# row counts are < 32768, so each serves as one int16 gather window for
# layer 2 (replacing the lo/hi split). Layout per section is rank-major.
SPLIT_T = 29


def _gpos_map(n_nodes, n_cores, tiles, npc):
    ra = SPLIT_T * P
    rb = npc - ra
    n = np.arange(n_nodes, dtype=np.int64)
    r = n // npc
    l = n % npc
    in_a = l < ra
    return np.where(in_a, r * ra + l, n_cores * ra + r * rb + (l - ra))


# ------------------------------------------------------------------ bass kernel
def _build(n_nodes, in_ch, hid, n_cores, lo_split, CLH1, CLH2, tiles, npc, prelu_a,
           reps=1, profile_mode=False, no_gather=False):
    import concourse.bacc as bacc
    import concourse.mybir as mybir
    import concourse.tile as tile

    from concourse.masks import make_identity

    dt = mybir.dt
    DT = dt.float16 if FP16 else dt.float32
    CL1, CH1 = CLH1
    CL2, CH2 = CLH2
    cpt1 = CL1 + CH1
    cpt2 = CL2 + CH2
    fb1 = in_ch // P   # feature blocks, layer-1 aggregation
    fb2 = hid // P
    last_rows = npc - (tiles - 1) * P
    ra = SPLIT_T * P        # section-A rows per rank
    rb = npc - ra           # section-B rows per rank

    nc = bacc.Bacc(
        "TRN2", target_bir_lowering=False, num_devices=n_cores,
        num_swdge_queues=NQ,
    )
    x16 = nc.dram_tensor("x16", [n_nodes, in_ch], DT, kind="ExternalInput")
    idx1 = nc.dram_tensor("idx1", [P, tiles * cpt1 * P // 16], dt.int16, kind="ExternalInput")
    meta1 = nc.dram_tensor("meta1", [P, tiles * cpt1 * 2], dt.float32, kind="ExternalInput")
    idx2 = nc.dram_tensor("idx2", [P, tiles * cpt2 * P // 16], dt.int16, kind="ExternalInput")
    meta2 = nc.dram_tensor("meta2", [P, tiles * cpt2 * 2], dt.float32, kind="ExternalInput")
    cnt1 = nc.dram_tensor("cnt1", [1, tiles * 2], dt.int32, kind="ExternalInput")
    cnt2 = nc.dram_tensor("cnt2", [1, tiles * 2], dt.int32, kind="ExternalInput")
    selfw = nc.dram_tensor("selfw", [P, tiles], dt.float32, kind="ExternalInput")
    xown = nc.dram_tensor("xown", [tiles * P, in_ch], DT, kind="ExternalInput")
    w1t = nc.dram_tensor("w1t", [P, fb1 * hid], DT, kind="ExternalInput")
    w2t = nc.dram_tensor("w2t", [P, fb2 * hid], DT, kind="ExternalInput")
    b1f = nc.dram_tensor("b1f", [1, hid], DT, kind="ExternalInput")
    b2f = nc.dram_tensor("b2f", [1, hid], DT, kind="ExternalInput")
    iota = nc.dram_tensor("iota", [P, P], DT, kind="ExternalInput")
    out = nc.dram_tensor("out", [npc, hid], dt.float32, kind="ExternalOutput")

    n_lo = min(lo_split, n_nodes)

    with tile.TileContext(nc) as tc:
        with (
            tc.tile_pool(name="const", bufs=1) as cpool,
            tc.tile_pool(name="work", bufs=WORK_BUFS) as pool,
            tc.tile_pool(name="psum", bufs=2, space="PSUM") as psum,
            tc.tile_pool(name="dram", bufs=1, space="DRAM") as dram,
        ):
            idx1_t = cpool.tile([P, tiles * cpt1 * P // 16], dt.int16)
            meta1_t = cpool.tile([P, tiles * cpt1 * 2], dt.float32)
            idx2_t = cpool.tile([P, tiles * cpt2 * P // 16], dt.int16)
            meta2_t = cpool.tile([P, tiles * cpt2 * 2], dt.float32)
            cnt1_t = cpool.tile([1, tiles * 2], dt.int32)
            cnt2_t = cpool.tile([1, tiles * 2], dt.int32)
            selfw_t = cpool.tile([P, tiles], dt.float32)
            hown = cpool.tile([P, tiles * hid], DT)
            w1_t = cpool.tile([P, fb1 * hid], DT)
            w2_t = cpool.tile([P, fb2 * hid], DT)
            b1_t = cpool.tile([1, hid], DT)
            b2_t = cpool.tile([1, hid], DT)
            ones_t = cpool.tile([1, P], DT)
            iota_t = cpool.tile([P, P], DT)
            ident_t = cpool.tile([P, P], DT)
            nc.sync.dma_start(idx1_t[:], idx1[:])
            nc.sync.dma_start(meta1_t[:], meta1[:])
            nc.sync.dma_start(idx2_t[:], idx2[:])
            nc.sync.dma_start(meta2_t[:], meta2[:])
            nc.sync.dma_start(cnt1_t[:], cnt1[:])
            nc.sync.dma_start(cnt2_t[:], cnt2[:])
            nc.sync.dma_start(selfw_t[:], selfw[:])
            nc.sync.dma_start(w1_t[:], w1t[:])
            nc.sync.dma_start(w2_t[:], w2t[:])
            nc.sync.dma_start(b1_t[:], b1f[:])
            nc.sync.dma_start(b2_t[:], b2f[:])
            nc.sync.dma_start(iota_t[:], iota[:])
            make_identity(nc, ident_t[:])
            nc.vector.memset(ones_t[:], 1.0)
            alpha_t = cpool.tile([P, 1], dt.float32)
            nc.vector.memset(alpha_t[:], float(prelu_a))
            creg = nc.gpsimd.alloc_register("gcnt") if NEG_PAD else None

            contribA = None
            contribB = None

            def layer(li, src_lo, src_hi, feat, fbk, CL, CH, idx_t, meta_t,
                      cnt_t, creg, wt_tile, bias_tile, dst_write, gt,
                      tile_done=None):
                """One GCN layer over all tiles of this core's dst range.

                Gathers are batched gt tiles per dma_gather (one lo + one hi
                call per group) to amortize the ~1us SWDGE fixed cost.
                src_lo/src_hi are the two int16-addressable gather windows.
                Aggregation: agg[dst, :] += S_ch.T.T @ G_ch with S stationary
                (one weight load per chunk, wide moving operand), then PE
                transpose to get the dense matmul's lhsT."""
                cpt = CL + CH
                for gi, t0 in enumerate(range(0, tiles, gt)):
                    t1 = min(t0 + gt, tiles)
                    gn = t1 - t0
                    glo = pool.tile([P, gn * CL, feat], DT, tag="glo", bufs=GBUFS)
                    ghi = pool.tile([P, max(gn * CH, 1), feat], DT,
                                    tag="ghi", bufs=GBUFS)
                    icol = t0 * cpt * P // 16
                    locol = gn * CL * P // 16
                    hicol = gn * CH * P // 16
                    # single_packet packs all of one engine's descriptors into
                    # one DMA packet (<=64 descs); batched gathers exceed that.
                    sp = gn * CL * P // 16 + 1 <= 64
                    if NEG_PAD and gi < GBUFS:
                        # first rotation of the 2-buffer g pool in each layer
                        # (layer-2 views are larger than layer-1's in the same
                        # slots): clear so slots skipped by negative-idx
                        # padding hold finite values (0 * NaN would poison
                        # the matmul)
                        nc.vector.memset(glo[:], 0)
                        nc.vector.memset(ghi[:], 0)
                    if not no_gather:
                        if NEG_PAD:
                            assert gn == 1
                            nc.gpsimd.reg_load(creg, cnt_t[0:1, 2 * t0 : 2 * t0 + 1])
                        nc.gpsimd.dma_gather(
                            glo[:],
                            src_lo,
                            idx_t[:, icol : icol + locol],
                            gn * CL * P,
                            creg if NEG_PAD else gn * CL * P,
                            feat,
                            queue_num=(2 * gi) % NQ,
                            single_packet=sp,
                        )
                        if CH > 0:
                            if NEG_PAD:
                                nc.gpsimd.reg_load(
                                    creg, cnt_t[0:1, 2 * t0 + 1 : 2 * t0 + 2]
                                )
                            nc.gpsimd.dma_gather(
                                ghi[:],
                                src_hi,
                                idx_t[:, icol + locol : icol + locol + hicol],
                                gn * CH * P,
                                creg if NEG_PAD else gn * CH * P,
                                feat,
                                queue_num=(2 * gi + 1) % NQ,
                                single_packet=sp,
                            )
                    for t in range(t0, t1):
                        ti = t - t0
                        rows = last_rows if t == tiles - 1 else P
                        s_t = pool.tile([P, cpt, P], DT, tag="s")
                        for ch in range(cpt):
                            m = t * cpt + ch
                            nc.vector.tensor_scalar(
                                out=s_t[:, ch, :],
                                in0=iota_t[:],
                                scalar1=meta_t[:, 2 * m : 2 * m + 1],
                                scalar2=meta_t[:, 2 * m + 1 : 2 * m + 2],
                                op0=mybir.AluOpType.is_equal,
                                op1=mybir.AluOpType.mult,
                            )
                        agg = psum.tile([P, feat], dt.float32, tag="agg")
                        for ch in range(cpt):
                            g_slice = (
                                glo[:, ti * CL + ch, :]
                                if ch < CL
                                else ghi[:, ti * CH + (ch - CL), :]
                            )
                            nc.tensor.matmul(
                                agg[:],
                                lhsT=s_t[:, ch, :],
                                rhs=g_slice,
                                start=(ch == 0),
                                stop=(ch == cpt - 1),
                            )
                        # self-loop term: agg[i] += dis^2[i] * own_feat[i]
                        # (contiguous own rows -> no gather needed)
                        if li == 1:
                            xo = pool.tile([P, feat], DT, tag="xo")
                            nc.sync.dma_start(xo[:], xown[t * P : (t + 1) * P, :])
                            own = xo[:]
                        else:
                            own = hown[:, t * hid : (t + 1) * hid]
                        nc.vector.scalar_tensor_tensor(
                            out=agg[:], in0=own,
                            scalar=selfw_t[:, t : t + 1], in1=agg[:],
                            op0=mybir.AluOpType.mult, op1=mybir.AluOpType.add,
                        )
                        agg_sb = pool.tile([P, feat], DT, tag="asb")
                        nc.scalar.copy(agg_sb[:], agg[:])
                        at_sb = pool.tile([P, fbk * P], DT, tag="at")
                        for f in range(fbk):
                            tp = psum.tile([P, P], DT, tag="tp")
                            nc.tensor.transpose(
                                tp[:], in_=agg_sb[:, f * P : (f + 1) * P],
                                identity=ident_t[:],
                            )
                            nc.scalar.copy(at_sb[:, f * P : (f + 1) * P], tp[:])
                        h_ps = psum.tile([P, hid], dt.float32, tag="hps")
                        # rank-1 bias: h_ps = ones.T @ b, then accumulate W
                        nc.tensor.matmul(
                            h_ps[:], lhsT=ones_t[:], rhs=bias_tile[:],
                            start=True, stop=False,
                        )
                        for f in range(fbk):
                            nc.tensor.matmul(
                                h_ps[:],
                                lhsT=at_sb[:, f * P : (f + 1) * P],
                                rhs=wt_tile[:, f * hid : (f + 1) * hid],
                                start=False,
                                stop=(f == fbk - 1),
                            )
                        dst_write(t, rows, h_ps)
                        if tile_done is not None:
                            tile_done(t)

            def write_h(t, rows, h_ps):
                # PReLU(x) = max(a*x, x) for 0<a<1: ACT copies PSUM->SBUF
                # (cast), then one fused DVE op. Output lands in the
                # persistent hown buffer (layer-2 self-loop source).
                h_sb = hown[:, t * hid : (t + 1) * hid]
                h_f = pool.tile([P, hid], DT, tag="hf")
                nc.scalar.copy(h_f[:], h_ps[:])
                nc.vector.scalar_tensor_tensor(
                    out=h_sb, in0=h_f[:], scalar=float(prelu_a), in1=h_f[:],
                    op0=mybir.AluOpType.mult, op1=mybir.AluOpType.max,
                )
                if t < SPLIT_T:
                    nc.sync.dma_start(
                        contribA[t * P : t * P + rows, :], h_sb[:rows, :]
                    )
                else:
                    roff = (t - SPLIT_T) * P
                    nc.sync.dma_start(
                        contribB[roff : roff + rows, :], h_sb[:rows, :]
                    )

            def write_out(t, rows, h_ps):
                o_sb = pool.tile([P, hid], dt.float32, tag="o2")
                nc.scalar.copy(o_sb[:], h_ps[:])
                nc.sync.dma_start(out[t * P : t * P + rows, :], o_sb[:rows, :])

            n_lo1 = min(lo_split, n_nodes)
            for _rep in range(reps):
                contribA = dram.tile([ra, hid], DT, tag=f"contribA_{_rep}",
                                     name=f"contribA_{_rep}")
                contribB = dram.tile([rb, hid], DT, tag=f"contribB_{_rep}",
                                     name=f"contribB_{_rep}")
                _aspace = "Shared" if SHARED_HFULL else "Local"
                hfullA = dram.tile([n_cores * ra, hid], DT, tag=f"hfullA_{_rep}",
                                   name=f"hfullA_{_rep}", addr_space=_aspace)
                hfullB = dram.tile([n_cores * rb, hid], DT, tag=f"hfullB_{_rep}",
                                   name=f"hfullB_{_rep}", addr_space=_aspace)

                def l1_tile_done(t):
                    # fire each section's AllGather as soon as its tiles are done
                    if profile_mode:
                        return
                    if t == SPLIT_T - 1:
                        nc.gpsimd.collective_compute(
                            "AllGather",
                            mybir.AluOpType.bypass,
                            replica_groups=[list(range(n_cores))],
                            ins=[contribA.opt()],
                            outs=[hfullA[:]],
                        )
                    elif t == tiles - 1:
                        nc.gpsimd.collective_compute(
                            "AllGather",
                            mybir.AluOpType.bypass,
                            replica_groups=[list(range(n_cores))],
                            ins=[contribB.opt()],
                            outs=[hfullB[:]],
                        )

                layer(1, x16[0:n_lo1, :], x16[n_lo1:n_nodes, :], in_ch, fb1,
                      CL1, CH1, idx1_t, meta1_t, cnt1_t, creg,
                      w1_t, b1_t, write_h, GT1, tile_done=l1_tile_done)
                layer(2, hfullA[:], hfullB[:], hid, fb2,
                      CL2, CH2, idx2_t, meta2_t, cnt2_t, creg,
                      w2_t, b2_t, write_out, GT2)
    nc.compile()
    return nc


# --------------------------------------------------------------------- runner
def _run(inputs, n_nodes, in_ch, hid, n_cores, lo_split):
    _apply_patches()
    from concourse.bass_utils import run_bass_kernel_spmd

    x = np.asarray(inputs["x"], np.float32)
    W1 = np.asarray(inputs["W1"], np.float32)
    W2 = np.asarray(inputs["W2"], np.float32)
    b1 = np.asarray(inputs["b1"], np.float32)
    b2 = np.asarray(inputs["b2"], np.float32)
    prelu_a = float(np.asarray(inputs["prelu_a"]))

    per_ct, tiles, npc, sw = _edge_buckets(
        inputs["edge_index"], inputs["edge_weight"], n_nodes, n_cores
    )
    gpos = _gpos_map(n_nodes, n_cores, tiles, npc)
    lo2 = n_cores * SPLIT_T * P  # section-A total rows = L2 window boundary
    idx1a, meta1a, cnt1a, CL1, CH1 = _pack(per_ct, n_cores, tiles, None, lo_split, GT1)
    idx2a, meta2a, cnt2a, CL2, CH2 = _pack(per_ct, n_cores, tiles, gpos, lo2, GT2)

    nc = _build(n_nodes, in_ch, hid, n_cores, lo_split, (CL1, CH1), (CL2, CH2),
                tiles, npc, prelu_a)

    npdt = np.float16 if FP16 else np.float32
    fb1 = in_ch // P
    fb2 = hid // P
    x16 = x.astype(npdt)
    w1t = W1.T.astype(npdt).reshape(fb1, P, hid).transpose(1, 0, 2).reshape(P, fb1 * hid)
    w2t = W2.T.astype(npdt).reshape(fb2, P, hid).transpose(1, 0, 2).reshape(P, fb2 * hid)
    b1f = b1[None, :].astype(npdt)
    b2f = b2[None, :].astype(npdt)
    iota = np.tile(np.arange(P, dtype=npdt)[None, :], (P, 1))
    sw_pad = np.zeros((tiles * P,), np.float32)
    xo_pad = np.zeros((tiles * P, in_ch), npdt)

    def core_self(c):
        s = sw_pad.copy()
        s[:npc] = sw[c * npc : (c + 1) * npc]
        xo = xo_pad.copy()
        xo[:npc] = x16[c * npc : (c + 1) * npc]
        return s.reshape(tiles, P).T.copy(), xo

    selfs = [core_self(c) for c in range(n_cores)]
    in_maps = [
        {
            "x16": x16,
            "idx1": idx1a[c], "meta1": meta1a[c],
            "idx2": idx2a[c], "meta2": meta2a[c],
            "cnt1": cnt1a[c], "cnt2": cnt2a[c],
            "selfw": selfs[c][0], "xown": selfs[c][1],
            "w1t": w1t, "w2t": w2t, "b1f": b1f, "b2f": b2f, "iota": iota,
        }
        for c in range(n_cores)
    ]
    res = run_bass_kernel_spmd(nc, in_maps, core_ids=list(range(n_cores)))
    outp = np.concatenate([res.results[c]["out"] for c in range(n_cores)], axis=0)
    return outp[:n_nodes]


def kernel(x, edge_index, edge_weight, W1, b1, W2, b2, prelu_a):
    inputs = dict(
        x=x, edge_index=edge_index, edge_weight=edge_weight,
        W1=W1, b1=b1, W2=W2, b2=b2, prelu_a=prelu_a,
    )
    return _run(inputs, N_NODES, IN_CH, HID, N_CORES, LO_SPLIT)


# ------------------------------------------------------------------- benchmark
def benchmark(inputs, n_iter=4, reps=(1, 5), profile_mode=False, no_gather=False):
    """Estimate pure device time of one kernel body via a replication delta:
    build the program with the body repeated r times; wall(r2) - wall(r1)
    cancels transfer/dispatch overhead. Returns ns per body."""
    import time
    _apply_patches()
    from concourse.bass_utils import run_bass_kernel_spmd

    x = np.asarray(inputs["x"], np.float32)
    prelu_a = float(np.asarray(inputs["prelu_a"]))
    per_ct, tiles, npc, sw = _edge_buckets(
        inputs["edge_index"], inputs["edge_weight"], N_NODES, N_CORES
    )
    gpos = _gpos_map(N_NODES, N_CORES, tiles, npc)
    lo2 = N_CORES * SPLIT_T * P
    idx1a, meta1a, cnt1a, CL1, CH1 = _pack(per_ct, N_CORES, tiles, None, LO_SPLIT, GT1)
    idx2a, meta2a, cnt2a, CL2, CH2 = _pack(per_ct, N_CORES, tiles, gpos, lo2, GT2)
    print(f"CL1={CL1} CH1={CH1} CL2={CL2} CH2={CH2}")
    npdt = np.float16 if FP16 else np.float32
    fb1, fb2 = IN_CH // P, HID // P
    W1 = np.asarray(inputs["W1"], np.float32)
    W2 = np.asarray(inputs["W2"], np.float32)
    x16 = x.astype(npdt)
    w1t = W1.T.astype(npdt).reshape(fb1, P, HID).transpose(1, 0, 2).reshape(P, fb1 * HID)
    w2t = W2.T.astype(npdt).reshape(fb2, P, HID).transpose(1, 0, 2).reshape(P, fb2 * HID)
    b1f = np.asarray(inputs["b1"], np.float32)[None, :].astype(npdt)
    b2f = np.asarray(inputs["b2"], np.float32)[None, :].astype(npdt)
    iota = np.tile(np.arange(P, dtype=npdt)[None, :], (P, 1))
    sw_pad = np.zeros((tiles * P,), np.float32)
    xo_pad = np.zeros((tiles * P, IN_CH), npdt)

    def core_self(c):
        s = sw_pad.copy()
        s[:npc] = sw[c * npc : (c + 1) * npc]
        xo = xo_pad.copy()
        xo[:npc] = x16[c * npc : (c + 1) * npc]
        return s.reshape(tiles, P).T.copy(), xo

    selfs = [core_self(c) for c in range(N_CORES)]
    in_maps = [
        {"x16": x16,
         "idx1": idx1a[c], "meta1": meta1a[c],
         "idx2": idx2a[c], "meta2": meta2a[c],
         "cnt1": cnt1a[c], "cnt2": cnt2a[c],
         "selfw": selfs[c][0], "xown": selfs[c][1],
         "w1t": w1t, "w2t": w2t, "b1f": b1f, "b2f": b2f, "iota": iota}
        for c in range(N_CORES)
    ]
    walls = {}
    for r in reps:
        nc = _build(N_NODES, IN_CH, HID, N_CORES, LO_SPLIT, (CL1, CH1), (CL2, CH2),
                    tiles, npc, prelu_a, reps=r, profile_mode=profile_mode,
                    no_gather=no_gather)
        ts = _timed_device_runs(nc, in_maps, n_iter)
        walls[r] = ts
        print(f"reps={r}: walls {['%.4f' % t for t in ts]}")
    r1, r2 = reps
    med = lambda v: sorted(v)[len(v) // 2]
    d = (med(walls[r2]) - med(walls[r1])) / (r2 - r1)
    return d * 1e9


def _timed_device_runs(nc, in_maps, n_iter):
    """Persistent-executable timed runs: inputs device-resident, outputs not
    fetched (block_until_ready only), so per-call wall ~= dispatch + exec."""
    import time
    import jax
    import jax.numpy as jnp
    from jax.sharding import Mesh, PartitionSpec, NamedSharding
    from jax.experimental.shard_map import shard_map
    import concourse.mybir as mybir
    from concourse.bass2jax import (
        install_neuronx_cc_hook, _bass_exec_p, partition_id_tensor,
    )

    install_neuronx_cc_hook()
    n_cores = len(in_maps)
    in_names, out_names, out_avals = [], [], []
    partition_name = nc.partition_id_tensor.name if nc.partition_id_tensor else None
    for alloc in nc.m.functions[0].allocations:
        if not isinstance(alloc, mybir.MemoryLocationSet):
            continue
        name = alloc.memorylocations[0].name
        if alloc.kind == "ExternalInput":
            if name != partition_name:
                in_names.append(name)
        elif alloc.kind == "ExternalOutput":
            out_names.append(name)
            out_avals.append(
                jax.core.ShapedArray(tuple(alloc.tensor_shape), mybir.dt.np(alloc.dtype))
            )
    n_params = len(in_names)
    all_in_names = in_names + out_names
    if partition_name is not None:
        all_in_names = all_in_names + [partition_name]

    def _body(*args):
        operands = list(args)
        if partition_name is not None:
            operands.append(partition_id_tensor())
        return tuple(
            _bass_exec_p.bind(
                *operands,
                out_avals=tuple(out_avals),
                in_names=tuple(all_in_names),
                out_names=tuple(out_names),
                lowering_input_output_aliases=(),
                sim_require_finite=True,
                sim_require_nnan=True,
                nc=nc,
            )
        )

    devices = jax.devices()[:n_cores]
    mesh = Mesh(np.asarray(devices), ("core",))
    spec = NamedSharding(mesh, PartitionSpec("core"))
    n_outs = len(out_names)
    donate = tuple(range(n_params, n_params + n_outs))
    sharded = jax.jit(
        shard_map(
            _body, mesh=mesh,
            in_specs=(PartitionSpec("core"),) * (n_params + n_outs),
            out_specs=(PartitionSpec("core"),) * n_outs,
            check_rep=False,
        ),
        donate_argnums=donate, keep_unused=True,
    )
    dev_in = [
        jax.device_put(
            np.concatenate([np.asarray(in_maps[c][nm]) for c in range(n_cores)], axis=0),
            spec,
        )
        for nm in in_names
    ]
    zero_shapes = [(n_cores * a.shape[0], *a.shape[1:]) for a in out_avals]

    def make_zeros():
        return [
            jax.device_put(jnp.zeros(s, a.dtype), spec)
            for s, a in zip(zero_shapes, out_avals)
        ]

    # Chained async timing: feed call k's outputs back as call k+1's donated
    # output buffers, block once at the end — dispatch overhead pipelines and
    # amortizes across the chain.
    n_chain = 20
    outs = tuple(make_zeros())
    outs = sharded(*dev_in, *outs)  # warmup + compile
    jax.block_until_ready(outs)
    ts = []
    for i in range(n_iter + 1):
        t0 = time.monotonic()
        for _ in range(n_chain):
            outs = sharded(*dev_in, *outs)
        jax.block_until_ready(outs)
        dt_s = (time.monotonic() - t0) / n_chain
        if i > 0:
            ts.append(dt_s)
    return ts



# revision 61
# speedup vs baseline: 1.2727x; 1.2727x over previous
"""Trainium2 Bass kernel for nn_CheckinEncoder (2-layer GCN, PReLU between).

Math (per GCNConv layer, PyG semantics):
    deg[d]  = sum_{e: dst_e=d} w_e + 1                (weighted in-degree + self loop)
    dis     = deg^{-1/2}
    norm_e  = dis[src_e] * w_e * dis[dst_e]           (self loop: 1/deg[d])
    agg     = scatter_add(norm_e * x[src_e] -> dst_e) (aggregate-first; linear
    h       = agg @ W.T + b                            and aggregation commute)

Sharding: dst nodes row-partitioned across 8 cores (6250 each). Each core
owns the edges into its nodes. Layer 1 gathers per-edge feature rows from a
replicated fp16 copy of x; between layers h is replicated via TWO
AllGathers into Shared-output DRAM tensors (A = tiles [0,28) fired
mid-layer-1 so it overlaps remaining compute, B at the end; single writer
per Shared tensor is required for the fast direct peer-write path). Each
section's row count is < 32768, so sections double as the two int16 gather
windows for layer 2 (layer 1 splits x at row 32768 instead).

Aggregation runs on the TensorEngine: for each 128-edge chunk a selection
matrix S[e, j] = (j == dst_local_e) * norm_e is built on the VectorEngine
(iota compare) and agg[dst, f] += S.T @ G accumulates in PSUM; a PE
transpose then yields the dense matmul's lhsT. The bias is folded into the
dense PSUM chain as a rank-1 (ones x b) matmul, and PReLU is one fused DVE
op after an ACT PSUM->SBUF copy.

Self loops never enter the gather: their contribution dis^2[i]*feat[i]
reads contiguous own rows (layer 1: strided DMA of the core's x slab;
layer 2: the persistent SBUF hown buffer written by layer 1) and is added
into the aggregation PSUM with one fused DVE op per tile.

Gather slots are padded to fixed per-(tile,half) chunk counts, but the
padding is never transferred: trailing idx slots hold -1 and the true
(x16-rounded) count is loaded into a GPSIMD register per call
(num_idxs_reg) — the DGE stops at the first negative index. The first
GBUFS rotations of the gather pool are memset so the untouched slots stay
finite (S weight 0 * NaN would poison the matmul).
"""

import numpy as np

# ---------------------------------------------------------------- problem dims
N_NODES = 50000
IN_CH = 256
HID = 512
N_CORES = 8
P = 128
LO_SPLIT = 32768
FP16 = True  # gather/matmul datapath dtype (fp32 accumulation throughout)
NQ = 4       # SWDGE queues used round-robin for gathers
WORK_BUFS = 3  # work tile pool depth (pipeline overlap)
GT1 = 1      # tiles per gather group (single_packet caps a call at 1008 idxs)
GT2 = 1
SHARED_HFULL = True  # Shared-output AllGather (direct peer writes)
GBUFS = 4    # gather tile buffers (in-flight gather depth)
NEG_PAD = True  # pad idx with -1: DGE stops at first negative -> no traffic
TINY_GATHER = False  # ablation: clamp gather counts to 16 rows (no real traffic)


# ------------------------------------------------------------------ tile patch
# This container's walrus accepts at most 1 sync wait per instruction
# (2 for EventSemaphore); Tile can emit more. Two fixes: split the kernel-tail
# drain's waits across nops, and legalize the final BIR by hoisting excess
# waits onto inserted NoOps (same engine, just before the instruction).
_PATCHED = False


def _apply_patches():
    global _PATCHED
    if _PATCHED:
        return
    _PATCHED = True
    import concourse.mybir as mybir
    import concourse.tile as tile
    import concourse.bass2jax as bass2jax
    import concourse.bass_utils as bass_utils
    from concourse.vector_clock import ScopedClock

    def _patched_drain_and_barrier(self, tick_clock, wait_clock):
        nc = self.nc
        drain_inst = nc.sync.drain()
        wait_clock.add_sem_waits(
            drain_inst.ins, ScopedClock({None: tick_clock.global_clock})
        )
        waits = list(drain_inst.ins.sync_info.on_wait)
        if len(waits) > 1:
            drain_inst.ins.sync_info.on_wait = waits[:1]
            for w in waits[1:]:
                nop = nc.sync.nop(nofuse=True, hint="drain_split_wait")
                if nop.ins.sync_info is None:
                    nop.ins.sync_info = mybir.SyncInfo(on_wait=[w], on_update=[])
                else:
                    nop.ins.sync_info.on_wait = [w]
        nc.all_engine_barrier()
        assert self.sems is not None
        popped = nc._tile_sem_poison_stack.pop()
        assert popped is self._sem_poison
        nc.clear_and_free_semaphores(list(self.sems.allocated().values()))
        nc.all_engine_barrier()

    tile.TileContext._drain_and_barrier = _patched_drain_and_barrier

    def _legalize_bir_json(bir_bytes):
        import orjson

        m = orjson.loads(bir_bytes)
        for fn in m.get("functions", []):
            for blk in fn.get("blocks", []):
                out = []
                for inst in blk.get("instructions", []):
                    si = inst.get("sync_info")
                    cap = 2 if inst.get("opcode") == "EventSemaphore" else 1
                    if si and len(si.get("on_wait") or []) > cap:
                        waits = si["on_wait"]
                        for k, w in enumerate(waits[:-cap]):
                            out.append(
                                {
                                    "debug": inst.get("debug", 0),
                                    "engine": inst["engine"],
                                    "ins": [],
                                    "outs": [],
                                    "name": f"{inst['name']}-lw{k}",
                                    "opcode": "NoOp",
                                    "sync_info": {"on_update": [], "on_wait": [w]},
                                }
                            )
                        si["on_wait"] = waits[-cap:]
                    out.append(inst)
                blk["instructions"] = out
        return orjson.dumps(m)

    orig = bass_utils.compile_bir_kernel

    def _wrapped(bir_json, tmpdir, neff_name="file.neff", **kw):
        return orig(_legalize_bir_json(bir_json), tmpdir, neff_name, **kw)

    bass_utils.compile_bir_kernel = _wrapped
    bass2jax.compile_bir_kernel = _wrapped


# ------------------------------------------------------------ host preprocessing
def _edge_buckets(edge_index, edge_weight, n_nodes, n_cores):
    """Bucket (src, dst_local, norm) per (core, tile). Self loops are NOT
    included: their contribution dis^2[i] * x[i] uses contiguous own rows and
    is applied on-device without the gather (see selfw)."""
    npc = n_nodes // n_cores
    tiles = (npc + P - 1) // P

    src = np.asarray(edge_index[0], dtype=np.int64)
    dst = np.asarray(edge_index[1], dtype=np.int64)
    w = np.asarray(edge_weight, dtype=np.float32)

    deg = np.bincount(dst, weights=w.astype(np.float64), minlength=n_nodes)
    deg = deg.astype(np.float32) + 1.0  # + self loop weight
    dis = 1.0 / np.sqrt(deg)
    norm = (dis[src] * w * dis[dst]).astype(np.float32)

    core_of = dst // npc
    tile_of = (dst % npc) // P
    dloc = (dst % npc) % P

    per_ct = {}
    for c in range(n_cores):
        cm = core_of == c
        for t in range(tiles):
            m = cm & (tile_of == t)
            per_ct[(c, t)] = (src[m], dloc[m], norm[m])
    selfw = dis * dis  # self-loop weight per node
    return per_ct, tiles, npc, selfw


def _pack(per_ct, n_cores, tiles, srcmap, lo_split, gt):
    """Pack bucketed edges into fixed-shape gather idx + (dst,norm) meta
    arrays. `srcmap` maps original src node id -> gather row id.

    idx layout is GROUP-major (gt tiles per gather group): for each group,
    all member tiles' lo indices are contiguous (CL*P slots per tile), then
    all hi indices (CH*P per tile) — one dma_gather per (group, half).
    meta stays per-tile: cols 2*(t*cpt+ch) with lo chunks first."""
    max_lo, max_hi = 1, 0
    split = {}
    for key, (s, d, n) in per_ct.items():
        g = srcmap[s] if srcmap is not None else s
        order = np.argsort(g, kind="stable")
        g, d2, n2 = g[order], d[order], n[order]
        lo = g < lo_split
        split[key] = (g[lo], d2[lo], n2[lo], g[~lo] - lo_split, d2[~lo], n2[~lo])
        max_lo = max(max_lo, int(lo.sum()))
        max_hi = max(max_hi, int((~lo).sum()))

    CL = (max_lo + P - 1) // P
    CH = (max_hi + P - 1) // P
    cpt = CL + CH

    idx_arrays, meta_arrays, cnt_arrays = [], [], []
    pad_idx = -1 if NEG_PAD else 0
    for c in range(n_cores):
        idx_flat = np.full((tiles * cpt * P,), pad_idx, np.int16)
        cnt = np.zeros((1, tiles * 2), np.int32)
        meta = np.zeros((P, tiles * cpt * 2), np.float32)
        for t0 in range(0, tiles, gt):
            t1 = min(t0 + gt, tiles)
            gbase = t0 * cpt * P
            lo_sz = (t1 - t0) * CL * P
            for t in range(t0, t1):
                ti = t - t0
                slo, dlo, nlo, shi, dhi, nhi = split[(c, t)]
                lo_off = gbase + ti * CL * P
                hi_off = gbase + lo_sz + ti * CH * P
                # round real counts up to x16 (DGE walks 16 lanes); pad the
                # rounded range with idx 0 (real gathers), -1 beyond (skipped)
                nlo16 = min(max((len(slo) + 15) // 16 * 16, 16), CL * P)
                nhi16 = min(max((len(shi) + 15) // 16 * 16, 16), CH * P)
                idx_flat[lo_off : lo_off + nlo16] = 0
                idx_flat[hi_off : hi_off + nhi16] = 0
                idx_flat[lo_off : lo_off + len(slo)] = slo.astype(np.int16)
                idx_flat[hi_off : hi_off + len(shi)] = shi.astype(np.int16)
                cnt[0, 2 * t] = 16 if TINY_GATHER else nlo16
                cnt[0, 2 * t + 1] = 16 if TINY_GATHER else nhi16
                dcol = np.zeros((cpt * P,), np.float32)
                ncol = np.zeros((cpt * P,), np.float32)
                dcol[: len(dlo)] = dlo
                ncol[: len(nlo)] = nlo
                dcol[CL * P : CL * P + len(dhi)] = dhi
                ncol[CL * P : CL * P + len(nhi)] = nhi
                for ch in range(cpt):
                    m = t * cpt + ch
                    meta[:, 2 * m] = dcol[ch * P : (ch + 1) * P]
                    meta[:, 2 * m + 1] = ncol[ch * P : (ch + 1) * P]
        idxw = idx_flat.reshape(-1, 16).T
        idx_arrays.append(np.tile(idxw, (8, 1)).copy())
        meta_arrays.append(meta)
        cnt_arrays.append(cnt)
    return idx_arrays, meta_arrays, cnt_arrays, CL, CH


# AllGather split: h is replicated via TWO AllGathers into two Shared-output
# DRAM tensors (single writer each — required for the fast shared-output
# collective path). Section A = tiles [0, SPLIT_T) fires mid-layer-1 and
# overlaps the rest of layer-1 compute; B fires at the end. Both sections'
# row counts are < 32768, so each serves as one int16 gather window for
# layer 2 (replacing the lo/hi split). Layout per section is rank-major.
SPLIT_T = 28


def _gpos_map(n_nodes, n_cores, tiles, npc):
    ra = SPLIT_T * P
    rb = npc - ra
    n = np.arange(n_nodes, dtype=np.int64)
    r = n // npc
    l = n % npc
    in_a = l < ra
    return np.where(in_a, r * ra + l, n_cores * ra + r * rb + (l - ra))


# ------------------------------------------------------------------ bass kernel
def _build(n_nodes, in_ch, hid, n_cores, lo_split, CLH1, CLH2, tiles, npc, prelu_a,
           reps=1, profile_mode=False, no_gather=False):
    import concourse.bacc as bacc
    import concourse.mybir as mybir
    import concourse.tile as tile

    from concourse.masks import make_identity

    dt = mybir.dt
    DT = dt.float16 if FP16 else dt.float32
    CL1, CH1 = CLH1
    CL2, CH2 = CLH2
    cpt1 = CL1 + CH1
    cpt2 = CL2 + CH2
    fb1 = in_ch // P   # feature blocks, layer-1 aggregation
    fb2 = hid // P
    last_rows = npc - (tiles - 1) * P
    ra = SPLIT_T * P        # section-A rows per rank
    rb = npc - ra           # section-B rows per rank

    nc = bacc.Bacc(
        "TRN2", target_bir_lowering=False, num_devices=n_cores,
        num_swdge_queues=NQ,
    )
    x16 = nc.dram_tensor("x16", [n_nodes, in_ch], DT, kind="ExternalInput")
    idx1 = nc.dram_tensor("idx1", [P, tiles * cpt1 * P // 16], dt.int16, kind="ExternalInput")
    meta1 = nc.dram_tensor("meta1", [P, tiles * cpt1 * 2], dt.float32, kind="ExternalInput")
    idx2 = nc.dram_tensor("idx2", [P, tiles * cpt2 * P // 16], dt.int16, kind="ExternalInput")
    meta2 = nc.dram_tensor("meta2", [P, tiles * cpt2 * 2], dt.float32, kind="ExternalInput")
    cnt1 = nc.dram_tensor("cnt1", [1, tiles * 2], dt.int32, kind="ExternalInput")
    cnt2 = nc.dram_tensor("cnt2", [1, tiles * 2], dt.int32, kind="ExternalInput")
    selfw = nc.dram_tensor("selfw", [P, tiles], dt.float32, kind="ExternalInput")
    xown = nc.dram_tensor("xown", [tiles * P, in_ch], DT, kind="ExternalInput")
    w1t = nc.dram_tensor("w1t", [P, fb1 * hid], DT, kind="ExternalInput")
    w2t = nc.dram_tensor("w2t", [P, fb2 * hid], DT, kind="ExternalInput")
    b1f = nc.dram_tensor("b1f", [1, hid], DT, kind="ExternalInput")
    b2f = nc.dram_tensor("b2f", [1, hid], DT, kind="ExternalInput")
    iota = nc.dram_tensor("iota", [P, P], DT, kind="ExternalInput")
    out = nc.dram_tensor("out", [npc, hid], dt.float32, kind="ExternalOutput")

    n_lo = min(lo_split, n_nodes)

    with tile.TileContext(nc) as tc:
        with (
            tc.tile_pool(name="const", bufs=1) as cpool,
            tc.tile_pool(name="work", bufs=WORK_BUFS) as pool,
            tc.tile_pool(name="psum", bufs=2, space="PSUM") as psum,
            tc.tile_pool(name="dram", bufs=1, space="DRAM") as dram,
        ):
            idx1_t = cpool.tile([P, tiles * cpt1 * P // 16], dt.int16)
            meta1_t = cpool.tile([P, tiles * cpt1 * 2], dt.float32)
            idx2_t = cpool.tile([P, tiles * cpt2 * P // 16], dt.int16)
            meta2_t = cpool.tile([P, tiles * cpt2 * 2], dt.float32)
            cnt1_t = cpool.tile([1, tiles * 2], dt.int32)
            cnt2_t = cpool.tile([1, tiles * 2], dt.int32)
            selfw_t = cpool.tile([P, tiles], dt.float32)
            hown = cpool.tile([P, tiles * hid], DT)
            w1_t = cpool.tile([P, fb1 * hid], DT)
            w2_t = cpool.tile([P, fb2 * hid], DT)
            b1_t = cpool.tile([1, hid], DT)
            b2_t = cpool.tile([1, hid], DT)
            ones_t = cpool.tile([1, P], DT)
            iota_t = cpool.tile([P, P], DT)
            ident_t = cpool.tile([P, P], DT)
            nc.sync.dma_start(idx1_t[:], idx1[:])
            nc.sync.dma_start(meta1_t[:], meta1[:])
            nc.sync.dma_start(idx2_t[:], idx2[:])
            nc.sync.dma_start(meta2_t[:], meta2[:])
            nc.sync.dma_start(cnt1_t[:], cnt1[:])
            nc.sync.dma_start(cnt2_t[:], cnt2[:])
            nc.sync.dma_start(selfw_t[:], selfw[:])
            nc.sync.dma_start(w1_t[:], w1t[:])
            nc.sync.dma_start(w2_t[:], w2t[:])
            nc.sync.dma_start(b1_t[:], b1f[:])
            nc.sync.dma_start(b2_t[:], b2f[:])
            nc.sync.dma_start(iota_t[:], iota[:])
            make_identity(nc, ident_t[:])
            nc.vector.memset(ones_t[:], 1.0)
            alpha_t = cpool.tile([P, 1], dt.float32)
            nc.vector.memset(alpha_t[:], float(prelu_a))
            creg = nc.gpsimd.alloc_register("gcnt") if NEG_PAD else None

            contribA = None
            contribB = None

            def layer(li, src_lo, src_hi, feat, fbk, CL, CH, idx_t, meta_t,
                      cnt_t, creg, wt_tile, bias_tile, dst_write, gt,
                      tile_done=None):
                """One GCN layer over all tiles of this core's dst range.

                Gathers are batched gt tiles per dma_gather (one lo + one hi
                call per group) to amortize the ~1us SWDGE fixed cost.
                src_lo/src_hi are the two int16-addressable gather windows.
                Aggregation: agg[dst, :] += S_ch.T.T @ G_ch with S stationary
                (one weight load per chunk, wide moving operand), then PE
                transpose to get the dense matmul's lhsT."""
                cpt = CL + CH
                for gi, t0 in enumerate(range(0, tiles, gt)):
                    t1 = min(t0 + gt, tiles)
                    gn = t1 - t0
                    glo = pool.tile([P, gn * CL, feat], DT, tag="glo", bufs=GBUFS)
                    ghi = pool.tile([P, max(gn * CH, 1), feat], DT,
                                    tag="ghi", bufs=GBUFS)
                    icol = t0 * cpt * P // 16
                    locol = gn * CL * P // 16
                    hicol = gn * CH * P // 16
                    # single_packet packs all of one engine's descriptors into
                    # one DMA packet (<=64 descs); batched gathers exceed that.
                    sp = gn * CL * P // 16 + 1 <= 64
                    if NEG_PAD and gi < GBUFS:
                        # first rotation of the 2-buffer g pool in each layer
                        # (layer-2 views are larger than layer-1's in the same
                        # slots): clear so slots skipped by negative-idx
                        # padding hold finite values (0 * NaN would poison
                        # the matmul)
                        nc.vector.memset(glo[:], 0)
                        nc.vector.memset(ghi[:], 0)
                    if not no_gather:
                        if NEG_PAD:
                            assert gn == 1
                            nc.gpsimd.reg_load(creg, cnt_t[0:1, 2 * t0 : 2 * t0 + 1])
                        nc.gpsimd.dma_gather(
                            glo[:],
                            src_lo,
                            idx_t[:, icol : icol + locol],
                            gn * CL * P,
                            creg if NEG_PAD else gn * CL * P,
                            feat,
                            queue_num=(2 * gi) % NQ,
                            single_packet=sp,
                        )
                        if CH > 0:
                            if NEG_PAD:
                                nc.gpsimd.reg_load(
                                    creg, cnt_t[0:1, 2 * t0 + 1 : 2 * t0 + 2]
                                )
                            nc.gpsimd.dma_gather(
                                ghi[:],
                                src_hi,
                                idx_t[:, icol + locol : icol + locol + hicol],
                                gn * CH * P,
                                creg if NEG_PAD else gn * CH * P,
                                feat,
                                queue_num=(2 * gi + 1) % NQ,
                                single_packet=sp,
                            )
                    for t in range(t0, t1):
                        ti = t - t0
                        rows = last_rows if t == tiles - 1 else P
                        s_t = pool.tile([P, cpt, P], DT, tag="s")
                        for ch in range(cpt):
                            m = t * cpt + ch
                            nc.vector.tensor_scalar(
                                out=s_t[:, ch, :],
                                in0=iota_t[:],
                                scalar1=meta_t[:, 2 * m : 2 * m + 1],
                                scalar2=meta_t[:, 2 * m + 1 : 2 * m + 2],
                                op0=mybir.AluOpType.is_equal,
                                op1=mybir.AluOpType.mult,
                            )
                        agg = psum.tile([P, feat], dt.float32, tag="agg")
                        for ch in range(cpt):
                            g_slice = (
                                glo[:, ti * CL + ch, :]
                                if ch < CL
                                else ghi[:, ti * CH + (ch - CL), :]
                            )
                            nc.tensor.matmul(
                                agg[:],
                                lhsT=s_t[:, ch, :],
                                rhs=g_slice,
                                start=(ch == 0),
                                stop=(ch == cpt - 1),
                            )
                        # self-loop term: agg[i] += dis^2[i] * own_feat[i]
                        # (contiguous own rows -> no gather needed)
                        if li == 1:
                            xo = pool.tile([P, feat], DT, tag="xo")
                            nc.sync.dma_start(xo[:], xown[t * P : (t + 1) * P, :])
                            own = xo[:]
                        else:
                            own = hown[:, t * hid : (t + 1) * hid]
                        nc.vector.scalar_tensor_tensor(
                            out=agg[:], in0=own,
                            scalar=selfw_t[:, t : t + 1], in1=agg[:],
                            op0=mybir.AluOpType.mult, op1=mybir.AluOpType.add,
                        )
                        agg_sb = pool.tile([P, feat], DT, tag="asb")
                        nc.scalar.copy(agg_sb[:], agg[:])
                        at_sb = pool.tile([P, fbk * P], DT, tag="at")
                        for f in range(fbk):
                            tp = psum.tile([P, P], DT, tag="tp")
                            nc.tensor.transpose(
                                tp[:], in_=agg_sb[:, f * P : (f + 1) * P],
                                identity=ident_t[:],
                            )
                            nc.scalar.copy(at_sb[:, f * P : (f + 1) * P], tp[:])
                        h_ps = psum.tile([P, hid], dt.float32, tag="hps")
                        # rank-1 bias: h_ps = ones.T @ b, then accumulate W
                        nc.tensor.matmul(
                            h_ps[:], lhsT=ones_t[:], rhs=bias_tile[:],
                            start=True, stop=False,
                        )
                        for f in range(fbk):
                            nc.tensor.matmul(
                                h_ps[:],
                                lhsT=at_sb[:, f * P : (f + 1) * P],
                                rhs=wt_tile[:, f * hid : (f + 1) * hid],
                                start=False,
                                stop=(f == fbk - 1),
                            )
                        dst_write(t, rows, h_ps)
                        if tile_done is not None:
                            tile_done(t)

            def write_h(t, rows, h_ps):
                # PReLU(x) = max(a*x, x) for 0<a<1: ACT copies PSUM->SBUF
                # (cast), then one fused DVE op. Output lands in the
                # persistent hown buffer (layer-2 self-loop source).
                h_sb = hown[:, t * hid : (t + 1) * hid]
                h_f = pool.tile([P, hid], DT, tag="hf")
                nc.scalar.copy(h_f[:], h_ps[:])
                nc.vector.scalar_tensor_tensor(
                    out=h_sb, in0=h_f[:], scalar=float(prelu_a), in1=h_f[:],
                    op0=mybir.AluOpType.mult, op1=mybir.AluOpType.max,
                )
                if t < SPLIT_T:
                    nc.sync.dma_start(
                        contribA[t * P : t * P + rows, :], h_sb[:rows, :]
                    )
                else:
                    roff = (t - SPLIT_T) * P
                    nc.sync.dma_start(
                        contribB[roff : roff + rows, :], h_sb[:rows, :]
                    )

            def write_out(t, rows, h_ps):
                o_sb = pool.tile([P, hid], dt.float32, tag="o2")
                nc.scalar.copy(o_sb[:], h_ps[:])
                nc.sync.dma_start(out[t * P : t * P + rows, :], o_sb[:rows, :])

            n_lo1 = min(lo_split, n_nodes)
            for _rep in range(reps):
                contribA = dram.tile([ra, hid], DT, tag=f"contribA_{_rep}",
                                     name=f"contribA_{_rep}")
                contribB = dram.tile([rb, hid], DT, tag=f"contribB_{_rep}",
                                     name=f"contribB_{_rep}")
                _aspace = "Shared" if SHARED_HFULL else "Local"
                hfullA = dram.tile([n_cores * ra, hid], DT, tag=f"hfullA_{_rep}",
                                   name=f"hfullA_{_rep}", addr_space=_aspace)
                hfullB = dram.tile([n_cores * rb, hid], DT, tag=f"hfullB_{_rep}",
                                   name=f"hfullB_{_rep}", addr_space=_aspace)

                def l1_tile_done(t):
                    # fire each section's AllGather as soon as its tiles are done
                    if profile_mode:
                        return
                    if t == SPLIT_T - 1:
                        nc.gpsimd.collective_compute(
                            "AllGather",
                            mybir.AluOpType.bypass,
                            replica_groups=[list(range(n_cores))],
                            ins=[contribA.opt()],
                            outs=[hfullA[:]],
                        )
                    elif t == tiles - 1:
                        nc.gpsimd.collective_compute(
                            "AllGather",
                            mybir.AluOpType.bypass,
                            replica_groups=[list(range(n_cores))],
                            ins=[contribB.opt()],
                            outs=[hfullB[:]],
                        )

                layer(1, x16[0:n_lo1, :], x16[n_lo1:n_nodes, :], in_ch, fb1,
                      CL1, CH1, idx1_t, meta1_t, cnt1_t, creg,
                      w1_t, b1_t, write_h, GT1, tile_done=l1_tile_done)
                layer(2, hfullA[:], hfullB[:], hid, fb2,
                      CL2, CH2, idx2_t, meta2_t, cnt2_t, creg,
                      w2_t, b2_t, write_out, GT2)
    nc.compile()
    return nc


# --------------------------------------------------------------------- runner
def _run(inputs, n_nodes, in_ch, hid, n_cores, lo_split):
    _apply_patches()
    from concourse.bass_utils import run_bass_kernel_spmd

    x = np.asarray(inputs["x"], np.float32)
    W1 = np.asarray(inputs["W1"], np.float32)
    W2 = np.asarray(inputs["W2"], np.float32)
    b1 = np.asarray(inputs["b1"], np.float32)
    b2 = np.asarray(inputs["b2"], np.float32)
    prelu_a = float(np.asarray(inputs["prelu_a"]))

    per_ct, tiles, npc, sw = _edge_buckets(
        inputs["edge_index"], inputs["edge_weight"], n_nodes, n_cores
    )
    gpos = _gpos_map(n_nodes, n_cores, tiles, npc)
    lo2 = n_cores * SPLIT_T * P  # section-A total rows = L2 window boundary
    idx1a, meta1a, cnt1a, CL1, CH1 = _pack(per_ct, n_cores, tiles, None, lo_split, GT1)
    idx2a, meta2a, cnt2a, CL2, CH2 = _pack(per_ct, n_cores, tiles, gpos, lo2, GT2)

    nc = _build(n_nodes, in_ch, hid, n_cores, lo_split, (CL1, CH1), (CL2, CH2),
                tiles, npc, prelu_a)

    npdt = np.float16 if FP16 else np.float32
    fb1 = in_ch // P
    fb2 = hid // P
    x16 = x.astype(npdt)
    w1t = W1.T.astype(npdt).reshape(fb1, P, hid).transpose(1, 0, 2).reshape(P, fb1 * hid)
    w2t = W2.T.astype(npdt).reshape(fb2, P, hid).transpose(1, 0, 2).reshape(P, fb2 * hid)
    b1f = b1[None, :].astype(npdt)
    b2f = b2[None, :].astype(npdt)
    iota = np.tile(np.arange(P, dtype=npdt)[None, :], (P, 1))
    sw_pad = np.zeros((tiles * P,), np.float32)
    xo_pad = np.zeros((tiles * P, in_ch), npdt)

    def core_self(c):
        s = sw_pad.copy()
        s[:npc] = sw[c * npc : (c + 1) * npc]
        xo = xo_pad.copy()
        xo[:npc] = x16[c * npc : (c + 1) * npc]
        return s.reshape(tiles, P).T.copy(), xo

    selfs = [core_self(c) for c in range(n_cores)]
    in_maps = [
        {
            "x16": x16,
            "idx1": idx1a[c], "meta1": meta1a[c],
            "idx2": idx2a[c], "meta2": meta2a[c],
            "cnt1": cnt1a[c], "cnt2": cnt2a[c],
            "selfw": selfs[c][0], "xown": selfs[c][1],
            "w1t": w1t, "w2t": w2t, "b1f": b1f, "b2f": b2f, "iota": iota,
        }
        for c in range(n_cores)
    ]
    res = run_bass_kernel_spmd(nc, in_maps, core_ids=list(range(n_cores)))
    outp = np.concatenate([res.results[c]["out"] for c in range(n_cores)], axis=0)
    return outp[:n_nodes]


def kernel(x, edge_index, edge_weight, W1, b1, W2, b2, prelu_a):
    inputs = dict(
        x=x, edge_index=edge_index, edge_weight=edge_weight,
        W1=W1, b1=b1, W2=W2, b2=b2, prelu_a=prelu_a,
    )
    return _run(inputs, N_NODES, IN_CH, HID, N_CORES, LO_SPLIT)


# ------------------------------------------------------------------- benchmark
def benchmark(inputs, n_iter=4, reps=(1, 5), profile_mode=False, no_gather=False):
    """Estimate pure device time of one kernel body via a replication delta:
    build the program with the body repeated r times; wall(r2) - wall(r1)
    cancels transfer/dispatch overhead. Returns ns per body."""
    import time
    _apply_patches()
    from concourse.bass_utils import run_bass_kernel_spmd

    x = np.asarray(inputs["x"], np.float32)
    prelu_a = float(np.asarray(inputs["prelu_a"]))
    per_ct, tiles, npc, sw = _edge_buckets(
        inputs["edge_index"], inputs["edge_weight"], N_NODES, N_CORES
    )
    gpos = _gpos_map(N_NODES, N_CORES, tiles, npc)
    lo2 = N_CORES * SPLIT_T * P
    idx1a, meta1a, cnt1a, CL1, CH1 = _pack(per_ct, N_CORES, tiles, None, LO_SPLIT, GT1)
    idx2a, meta2a, cnt2a, CL2, CH2 = _pack(per_ct, N_CORES, tiles, gpos, lo2, GT2)
    print(f"CL1={CL1} CH1={CH1} CL2={CL2} CH2={CH2}")
    npdt = np.float16 if FP16 else np.float32
    fb1, fb2 = IN_CH // P, HID // P
    W1 = np.asarray(inputs["W1"], np.float32)
    W2 = np.asarray(inputs["W2"], np.float32)
    x16 = x.astype(npdt)
    w1t = W1.T.astype(npdt).reshape(fb1, P, HID).transpose(1, 0, 2).reshape(P, fb1 * HID)
    w2t = W2.T.astype(npdt).reshape(fb2, P, HID).transpose(1, 0, 2).reshape(P, fb2 * HID)
    b1f = np.asarray(inputs["b1"], np.float32)[None, :].astype(npdt)
    b2f = np.asarray(inputs["b2"], np.float32)[None, :].astype(npdt)
    iota = np.tile(np.arange(P, dtype=npdt)[None, :], (P, 1))
    sw_pad = np.zeros((tiles * P,), np.float32)
    xo_pad = np.zeros((tiles * P, IN_CH), npdt)

    def core_self(c):
        s = sw_pad.copy()
        s[:npc] = sw[c * npc : (c + 1) * npc]
        xo = xo_pad.copy()
        xo[:npc] = x16[c * npc : (c + 1) * npc]
        return s.reshape(tiles, P).T.copy(), xo

    selfs = [core_self(c) for c in range(N_CORES)]
    in_maps = [
        {"x16": x16,
         "idx1": idx1a[c], "meta1": meta1a[c],
         "idx2": idx2a[c], "meta2": meta2a[c],
         "cnt1": cnt1a[c], "cnt2": cnt2a[c],
         "selfw": selfs[c][0], "xown": selfs[c][1],
         "w1t": w1t, "w2t": w2t, "b1f": b1f, "b2f": b2f, "iota": iota}
        for c in range(N_CORES)
    ]
    walls = {}
    for r in reps:
        nc = _build(N_NODES, IN_CH, HID, N_CORES, LO_SPLIT, (CL1, CH1), (CL2, CH2),
                    tiles, npc, prelu_a, reps=r, profile_mode=profile_mode,
                    no_gather=no_gather)
        ts = _timed_device_runs(nc, in_maps, n_iter)
        walls[r] = ts
        print(f"reps={r}: walls {['%.4f' % t for t in ts]}")
    r1, r2 = reps
    med = lambda v: sorted(v)[len(v) // 2]
    d = (med(walls[r2]) - med(walls[r1])) / (r2 - r1)
    return d * 1e9


def _timed_device_runs(nc, in_maps, n_iter):
    """Persistent-executable timed runs: inputs device-resident, outputs not
    fetched (block_until_ready only), so per-call wall ~= dispatch + exec."""
    import time
    import jax
    import jax.numpy as jnp
    from jax.sharding import Mesh, PartitionSpec, NamedSharding
    from jax.experimental.shard_map import shard_map
    import concourse.mybir as mybir
    from concourse.bass2jax import (
        install_neuronx_cc_hook, _bass_exec_p, partition_id_tensor,
    )

    install_neuronx_cc_hook()
    n_cores = len(in_maps)
    in_names, out_names, out_avals = [], [], []
    partition_name = nc.partition_id_tensor.name if nc.partition_id_tensor else None
    for alloc in nc.m.functions[0].allocations:
        if not isinstance(alloc, mybir.MemoryLocationSet):
            continue
        name = alloc.memorylocations[0].name
        if alloc.kind == "ExternalInput":
            if name != partition_name:
                in_names.append(name)
        elif alloc.kind == "ExternalOutput":
            out_names.append(name)
            out_avals.append(
                jax.core.ShapedArray(tuple(alloc.tensor_shape), mybir.dt.np(alloc.dtype))
            )
    n_params = len(in_names)
    all_in_names = in_names + out_names
    if partition_name is not None:
        all_in_names = all_in_names + [partition_name]

    def _body(*args):
        operands = list(args)
        if partition_name is not None:
            operands.append(partition_id_tensor())
        return tuple(
            _bass_exec_p.bind(
                *operands,
                out_avals=tuple(out_avals),
                in_names=tuple(all_in_names),
                out_names=tuple(out_names),
                lowering_input_output_aliases=(),
                sim_require_finite=True,
                sim_require_nnan=True,
                nc=nc,
            )
        )

    devices = jax.devices()[:n_cores]
    mesh = Mesh(np.asarray(devices), ("core",))
    spec = NamedSharding(mesh, PartitionSpec("core"))
    n_outs = len(out_names)
    donate = tuple(range(n_params, n_params + n_outs))
    sharded = jax.jit(
        shard_map(
            _body, mesh=mesh,
            in_specs=(PartitionSpec("core"),) * (n_params + n_outs),
            out_specs=(PartitionSpec("core"),) * n_outs,
            check_rep=False,
        ),
        donate_argnums=donate, keep_unused=True,
    )
    dev_in = [
        jax.device_put(
            np.concatenate([np.asarray(in_maps[c][nm]) for c in range(n_cores)], axis=0),
            spec,
        )
        for nm in in_names
    ]
    zero_shapes = [(n_cores * a.shape[0], *a.shape[1:]) for a in out_avals]

    def make_zeros():
        return [
            jax.device_put(jnp.zeros(s, a.dtype), spec)
            for s, a in zip(zero_shapes, out_avals)
        ]

    # Chained async timing: feed call k's outputs back as call k+1's donated
    # output buffers, block once at the end — dispatch overhead pipelines and
    # amortizes across the chain.
    n_chain = 20
    outs = tuple(make_zeros())
    outs = sharded(*dev_in, *outs)  # warmup + compile
    jax.block_until_ready(outs)
    ts = []
    for i in range(n_iter + 1):
        t0 = time.monotonic()
        for _ in range(n_chain):
            outs = sharded(*dev_in, *outs)
        jax.block_until_ready(outs)
        dt_s = (time.monotonic() - t0) / n_chain
        if i > 0:
            ts.append(dt_s)
    return ts

